# revision 2
# baseline (speedup 1.0000x reference)
"""TRN2 Bass kernel for nn_HCSMoEQwen3MoeSparseMoeBlock (8-core expert-parallel).

Reference semantics: router softmax over 32 experts -> top-8 -> normalized
per-(token,group) weights via merge_groups; every token is processed by the 8
groups' dominant experts (SwiGLU MLPs); outputs combined with the weights.

Sharding: core g owns group g's dominant expert (gate_up/down weights for
that expert only) and processes ALL tokens; the router is replicated (each
core computes only its own group's combined weight w_g[t]). Each core returns
w_g[t] * y_g[t, :]; the host sums the 8 partial outputs.

Per-core dataflow (all matmul operands float32r: full PE rate, ~1.5e-4 rel):
  for each 128-token chunk:
    h[t,0:1536] + logits[t,0:32] = xT-chunk.T @ [gu_gT | gwT]   (PE, K=2048)
    top-8 of logits per token -> w_g[t]                          (DVE/ACT)
    act[t,0:768] = silu(h[:,:768]) * h[:,768:]                   (ACT/DVE)
    actT = PE-transpose(act)                                     (PE)
    y[t,:] = actT.T @ dnT                                        (PE, K=768)
    out[t,:] = w_g[t] * y[t,:] -> DRAM                           (ACT)
"""
import numpy as np

import concourse.bass as bass
import concourse.mybir as mybir
import concourse.tile as tile
from concourse import bacc
from concourse.bass_utils import run_bass_kernel_spmd
from concourse.masks import make_identity

T = 2048          # tokens
H = 2048          # hidden
I2 = 1536         # 2 * intermediate
I = 768           # intermediate
E = 32            # experts
G = 8             # groups / cores
TOP_K = 8
KO = H // 128     # 16 k-subtiles for the H contraction
JO = I // 128     # 6 k-subtiles for the I contraction
TCH = 128         # token chunk
NCHUNK = T // TCH # 16
HB = 512          # h/output column chunk
NEG_BIG = -1.0e9

F32 = mybir.dt.float32
F32R = mybir.dt.float32r
AX = mybir.AxisListType.X
OP = mybir.AluOpType
ACTF = mybir.ActivationFunctionType

_CACHED_NC = None


def _build():
    global _CACHED_NC
    if _CACHED_NC is not None:
        return _CACHED_NC
    nc = bacc.Bacc("TRN2", target_bir_lowering=False, debug=False, num_devices=G)

    xT_d = nc.dram_tensor("xT", [H, T], F32R, kind="ExternalInput")
    gugw_d = nc.dram_tensor("gugw", [H, I2 + E], F32R, kind="ExternalInput")
    dnT_d = nc.dram_tensor("dnT", [I, H], F32R, kind="ExternalInput")
    mgb_d = nc.dram_tensor("mgb", [128, E], F32, kind="ExternalInput")
    y_d = nc.dram_tensor("y", [T, H], F32, kind="ExternalOutput")

    xT_ap = xT_d.ap().rearrange("(ko p) t -> p ko t", p=128)
    gugw_ap = gugw_d.ap().rearrange("(ko p) o -> p ko o", p=128)
    dnT_ap = dnT_d.ap().rearrange("(jo p) h -> p jo h", p=128)

    with tile.TileContext(nc) as tc:
        with (
            tc.tile_pool(name="const", bufs=1) as cpool,
            tc.tile_pool(name="weights", bufs=1) as wpool,
            tc.tile_pool(name="xin", bufs=2) as xpool,
            tc.tile_pool(name="acts", bufs=2) as apool,
            tc.tile_pool(name="router", bufs=2) as rpool,
            tc.tile_pool(name="yout", bufs=3) as ypool,
            tc.tile_pool(name="ph", bufs=1, space="PSUM") as pph,
            tc.tile_pool(name="ps", bufs=1, space="PSUM") as pps,
            tc.tile_pool(name="py", bufs=2, space="PSUM") as ppy,
        ):
            identity = cpool.tile([128, 128], F32, tag="identity")
            make_identity(nc, identity)
            negbig = cpool.tile([128, E], F32, tag="negbig")
            nc.vector.memset(negbig, NEG_BIG)
            mgb_sb = cpool.tile([128, E], F32, tag="mgb")
            nc.sync.dma_start(mgb_sb[:], mgb_d.ap())

            gugw_sb = wpool.tile([128, KO, I2 + E], F32R, tag="gugw")
            nc.sync.dma_start(gugw_sb[:], gugw_ap)
            dn_sb = wpool.tile([128, JO, H], F32R, tag="dn")
            nc.sync.dma_start(dn_sb[:], dnT_ap)

            for tci in range(NCHUNK):
                tsl = slice(tci * TCH, (tci + 1) * TCH)

                xT_c = xpool.tile([128, KO, TCH], F32R, tag="xT_c")
                nc.sync.dma_start(xT_c[:], xT_ap[:, :, tsl])

                # ---- M1: h (3x512 cols) + router logits (32 cols) ----
                h_ps = pph.tile([128, 3, HB], F32, tag="h_ps")
                s_ps = pps.tile([128, 8, 128], F32, tag="s_ps")
                for k in range(KO):
                    st, sp = (k == 0), (k == KO - 1)
                    nc.tensor.matmul(
                        s_ps[:, 0, :E], xT_c[:, k], gugw_sb[:, k, I2:],
                        start=st, stop=sp,
                    )
                    for b in range(3):
                        nc.tensor.matmul(
                            h_ps[:, b], xT_c[:, k],
                            gugw_sb[:, k, b * HB:(b + 1) * HB],
                            start=st, stop=sp,
                        )

                # ---- router: top-8 -> per-token group weight ----
                logits = rpool.tile([128, E], F32, tag="logits")
                nc.scalar.copy(logits[:], s_ps[:, 0, :E])
                cur = rpool.tile([128, E], F32, tag="cur")
                nc.vector.tensor_copy(cur[:], logits[:])
                msk = rpool.tile([128, E], mybir.dt.uint8, tag="msk")
                m1 = rpool.tile([128, 1], F32, tag="m1")
                mk = rpool.tile([128, 1], F32, tag="mk")
                for it in range(TOP_K - 1):
                    tgt = m1 if it == 0 else mk
                    nc.vector.reduce_max(tgt[:], cur[:], axis=AX)
                    nc.vector.tensor_scalar(msk[:], cur[:], tgt[:], None, OP.is_ge)
                    nc.vector.copy_predicated(cur[:], msk[:], negbig[:])
                m8 = rpool.tile([128, 1], F32, tag="m8")
                nc.vector.reduce_max(m8[:], cur[:], axis=AX)

                nm1 = rpool.tile([128, 1], F32, tag="nm1")
                nc.vector.tensor_scalar(nm1[:], m1[:], -1.0, None, OP.mult)
                mask8 = rpool.tile([128, E], F32, tag="mask8")
                nc.vector.tensor_scalar(mask8[:], logits[:], m8[:], None, OP.is_ge)
                ew = rpool.tile([128, E], F32, tag="ew")
                nc.scalar.activation(ew[:], logits[:], ACTF.Exp, bias=nm1[:])
                nc.vector.tensor_tensor(ew[:], ew[:], mask8[:], OP.mult)
                s8 = rpool.tile([128, 1], F32, tag="s8")
                nc.vector.reduce_sum(s8[:], ew[:], axis=AX)
                nc.vector.tensor_tensor(ew[:], ew[:], mgb_sb[:], OP.mult)
                num = rpool.tile([128, 1], F32, tag="num")
                nc.vector.reduce_sum(num[:], ew[:], axis=AX)
                rs = rpool.tile([128, 1], F32, tag="rs")
                nc.vector.reciprocal(rs[:], s8[:])
                w_t = rpool.tile([128, 1], F32, tag="w_t")
                nc.vector.tensor_tensor(w_t[:], num[:], rs[:], OP.mult)

                # ---- SwiGLU: act = silu(h[:, :768]) * h[:, 768:1536] ----
                silu_sb = apool.tile([128, I], F32, tag="silu")
                nc.scalar.activation(silu_sb[:, :HB], h_ps[:, 0], ACTF.Silu)
                nc.scalar.activation(silu_sb[:, HB:I], h_ps[:, 1, :I - HB], ACTF.Silu)
                act_sb = apool.tile([128, I], F32, tag="act")
                nc.vector.tensor_tensor(
                    act_sb[:, :I - HB], silu_sb[:, :I - HB],
                    h_ps[:, 1, I - HB:], OP.mult,
                )
                nc.vector.tensor_tensor(
                    act_sb[:, I - HB:], silu_sb[:, I - HB:],
                    h_ps[:, 2], OP.mult,
                )

                # ---- transpose act -> actT (PE) ----
                actT_sb = apool.tile([128, JO, TCH], F32R, tag="actT")
                for j in range(JO):
                    nc.tensor.transpose(
                        s_ps[:, 2 + j], act_sb[:, j * 128:(j + 1) * 128], identity,
                    )
                    nc.scalar.copy(actT_sb[:, j], s_ps[:, 2 + j])

                # ---- M2: y = actT.T @ dnT, scaled by w ----
                for hb in range(H // HB):
                    y_ps = ppy.tile([128, HB], F32, tag="y_ps")
                    for j in range(JO):
                        nc.tensor.matmul(
                            y_ps[:], actT_sb[:, j],
                            dn_sb[:, j, hb * HB:(hb + 1) * HB],
                            start=(j == 0), stop=(j == JO - 1),
                        )
                    y_sb = ypool.tile([128, HB], F32, tag="y_sb")
                    nc.scalar.mul(y_sb[:], y_ps[:], w_t[:])
                    nc.sync.dma_start(
                        y_d.ap()[tsl, hb * HB:(hb + 1) * HB], y_sb[:],
                    )
    nc.compile()
    _CACHED_NC = nc
    return nc


def prepare_in_maps(hidden_states, gate_weight, gate_up_proj, down_proj,
                    merge_groups, dominant_experts):
    x = np.asarray(hidden_states, dtype=np.float32).reshape(T, H)
    xT = np.ascontiguousarray(x.T)
    gw = np.asarray(gate_weight, dtype=np.float32)
    gwT = np.ascontiguousarray(gw.T)  # [H, E]
    mg = np.asarray(merge_groups).astype(np.int64)
    de = np.asarray(dominant_experts).astype(np.int64)
    gup = np.asarray(gate_up_proj, dtype=np.float32)
    dnp_ = np.asarray(down_proj, dtype=np.float32)

    in_maps = []
    for g in range(G):
        e = int(de[g])
        gugw = np.empty((H, I2 + E), dtype=np.float32)
        gugw[:, :I2] = gup[e].T
        gugw[:, I2:] = gwT
        dnT = np.ascontiguousarray(dnp_[e].T)  # [I, H]
        mgb = np.ascontiguousarray(
            np.broadcast_to((mg == g).astype(np.float32)[None, :], (128, E))
        )
        in_maps.append({"xT": xT, "gugw": gugw, "dnT": dnT, "mgb": mgb})
    return in_maps


def kernel(hidden_states, gate_weight, gate_up_proj, down_proj,
           merge_groups, dominant_experts):
    in_maps = prepare_in_maps(hidden_states, gate_weight, gate_up_proj,
                              down_proj, merge_groups, dominant_experts)
    nc = _build()
    res = run_bass_kernel_spmd(nc, in_maps, core_ids=list(range(G)), trace=False)
    out = np.zeros((T, H), dtype=np.float64)
    for r in res.results:
        out += r["y"].astype(np.float64)
    return out.astype(np.float32).reshape(1, T, H)
